# revision 8
# baseline (speedup 1.0000x reference)
"""Trainium2 Bass kernel for nn_AttnReadout (segment attention readout).

Computation (reference):
    anchor[b]  = mean of ifeat rows in segment b                  [B, D]
    e[i]       = sigmoid(ifeat @ Wu.T + (anchor @ Wv.T + bv)[seg]) @ we
    alpha      = segment_softmax(e)
    rst[b]     = sum_i alpha[i] * ifeat[i]                        [B, D]
    out        = concat([rst, anchor], axis=1)                    [B, 2D]

Sharding: 2048 segments -> 8 cores x 2 windows of 128 contiguous segments.
Nodes (sorted by segment) are padded per-window to T_W tiles of 128 rows.
All segment reductions are one-hot matmuls on the tensor engine; gathers of
per-segment vectors are one-hot-transposed matmuls.
"""

import numpy as np

N = 102400
D = 256
B = 2048
N_CORES = 8
W_PER_CORE = 2
N_WINDOWS = N_CORES * W_PER_CORE  # 16
SEGS_PER_WINDOW = B // N_WINDOWS  # 128
P = 128


def _apply_tile_patch():
    """Split TileContext's multi-wait tail drain into single-wait drains.

    This walrus build rejects >1 sync wait on a CTRL (Drain) instruction
    ("Too many sync wait commands"); a chain of single-wait drains on SP is
    semantically identical.
    """
    import concourse.tile as tile_mod
    from concourse.vector_clock import ScopedClock

    if getattr(tile_mod.TileContext, "_drain_wait_split_patch", False):
        return

    def _patched(self, tick_clock, wait_clock):
        nc = self.nc
        drain_inst = nc.sync.drain()
        wait_clock.add_sem_waits(
            drain_inst.ins, ScopedClock({None: tick_clock.global_clock})
        )
        si = drain_inst.ins.sync_info
        waits = list(si.on_wait)
        if len(waits) > 1:
            SyncInfo = type(si)
            drain_inst.ins.sync_info = SyncInfo(
                on_wait=[waits[0]], on_update=list(si.on_update)
            )
            for w in waits[1:]:
                extra = nc.sync.drain()
                extra.ins.sync_info = SyncInfo(on_wait=[w], on_update=[])

        nc.all_engine_barrier()
        assert self.sems is not None
        popped = nc._tile_sem_poison_stack.pop()
        assert popped is self._sem_poison
        nc.clear_and_free_semaphores(list(self.sems.allocated().values()))
        nc.all_engine_barrier()

    tile_mod.TileContext._drain_and_barrier = _patched
    tile_mod.TileContext._drain_wait_split_patch = True


def _split_sync_waits(nc, limit=1):
    """Split >limit sync waits per instruction into preceding single-wait
    EventSemaphore carriers on the same engine (walrus build limit)."""
    import concourse.mybir as mybir

    n_new = 0
    for _, bassbb in nc.bb_map.items():
        insts = bassbb.bb.instructions  # live list
        snapshot = list(insts)
        offset = 0
        for pos, inst in enumerate(snapshot):
            si = getattr(inst, "sync_info", None)
            if si is None:
                continue
            waits = list(si.on_wait)
            if len(waits) <= limit:
                continue
            SyncInfo = type(si)
            inst.sync_info = SyncInfo(
                on_wait=waits[:limit], on_update=list(si.on_update))
            carriers = []
            for w in waits[limit:]:
                c = mybir.InstEventSemaphore(
                    name=f"WSPLIT-{nc.next_id()}", ins=[], outs=[])
                c.engine = inst.engine
                c.sync_info = SyncInfo(on_wait=[w], on_update=[])
                carriers.append(c)
            insts[pos + offset:pos + offset] = carriers
            offset += len(carriers)
            n_new += len(carriers)
    return n_new


def _build(T_W):
    """Build the single-core SPMD Bass program for T_W 128-node tiles/window."""
    import concourse.bass as bass
    import concourse.mybir as mybir
    from concourse.tile import TileContext

    _apply_tile_patch()

    f32 = mybir.dt.float32
    Alu = mybir.AluOpType
    Act = mybir.ActivationFunctionType

    NT = W_PER_CORE * T_W  # total node tiles per core

    nc = bass.Bass("TRN2", num_devices=N_CORES)

    if_dram = nc.dram_tensor("ifeat", [NT, P, D + 1], f32, kind="ExternalInput")
    seg_dram = nc.dram_tensor("seg", [NT, P, 1], f32, kind="ExternalInput")
    wuT_dram = nc.dram_tensor("wuT", [2, P, D], f32, kind="ExternalInput")
    wvT_dram = nc.dram_tensor("wvT", [2, P, D], f32, kind="ExternalInput")
    web_dram = nc.dram_tensor("web", [P, D], f32, kind="ExternalInput")
    bvb_dram = nc.dram_tensor("bvb", [P, D], f32, kind="ExternalInput")
    ident_dram = nc.dram_tensor("ident", [P, P], f32, kind="ExternalInput")
    iota_dram = nc.dram_tensor("iota", [P, P], f32, kind="ExternalInput")
    out_dram = nc.dram_tensor("out", [W_PER_CORE, P, 2 * D], f32,
                              kind="ExternalOutput")

    with TileContext(nc) as tc:
        import contextlib

        with contextlib.ExitStack() as ctx:
            const_pool = ctx.enter_context(tc.tile_pool(name="const", bufs=1))
            nat_pool = ctx.enter_context(tc.tile_pool(name="nat", bufs=1))
            seg_pool = ctx.enter_context(tc.tile_pool(name="segp", bufs=1))
            oh_pool = ctx.enter_context(tc.tile_pool(name="oh", bufs=3))
            ohT_pool = ctx.enter_context(tc.tile_pool(name="ohT", bufs=2))
            ifT_pool = ctx.enter_context(tc.tile_pool(name="ifT", bufs=2))
            s_pool = ctx.enter_context(tc.tile_pool(name="s", bufs=2))
            prod_pool = ctx.enter_context(tc.tile_pool(name="prod", bufs=2))
            zx_pool = ctx.enter_context(tc.tile_pool(name="zx", bufs=2))
            col_pool = ctx.enter_context(tc.tile_pool(name="col", bufs=4))
            wnd_pool = ctx.enter_context(tc.tile_pool(name="wnd", bufs=2))
            anchor_ps_pool = ctx.enter_context(
                tc.tile_pool(name="anchor_ps", bufs=1, space="PSUM"))
            wsum_ps_pool = ctx.enter_context(
                tc.tile_pool(name="wsum_ps", bufs=1, space="PSUM"))
            tr_ps_pool = ctx.enter_context(
                tc.tile_pool(name="tr_ps", bufs=2, space="PSUM"))
            s_ps_pool = ctx.enter_context(
                tc.tile_pool(name="s_ps", bufs=2, space="PSUM"))
            fv_ps_pool = ctx.enter_context(
                tc.tile_pool(name="fv_ps", bufs=1, space="PSUM"))

            # constants
            wuT_sb = const_pool.tile([P, 2, D], f32, name="wuT_sb", tag="wuT_sb")
            nc.sync.dma_start(wuT_sb[:], wuT_dram[:].rearrange("k p d -> p k d"))
            wvT_sb = const_pool.tile([P, 2, D], f32, name="wvT_sb", tag="wvT_sb")
            nc.sync.dma_start(wvT_sb[:], wvT_dram[:].rearrange("k p d -> p k d"))
            web_sb = const_pool.tile([P, D], f32, name="web_sb", tag="web_sb")
            nc.sync.dma_start(web_sb[:], web_dram[:])
            bvb_sb = const_pool.tile([P, D], f32, name="bvb_sb", tag="bvb_sb")
            nc.sync.dma_start(bvb_sb[:], bvb_dram[:])
            ident_sb = const_pool.tile([P, P], f32, name="ident_sb", tag="ident_sb")
            nc.sync.dma_start(ident_sb[:], ident_dram[:])
            iota_sb = const_pool.tile([P, P], f32, name="iota_sb", tag="iota_sb")
            nc.sync.dma_start(iota_sb[:], iota_dram[:])

            # resident per-tile node data
            nat_tiles = []
            seg_tiles = []
            for g in range(NT):
                nat = nat_pool.tile([P, D + 1], f32, name=f"nat{g}", tag=f"nat{g}")
                nc.sync.dma_start(nat[:], if_dram[g])
                nat_tiles.append(nat)
                sg = seg_pool.tile([P, 1], f32, name=f"segc{g}", tag=f"segc{g}")
                nc.sync.dma_start(sg[:], seg_dram[g])
                seg_tiles.append(sg)

            for w in range(W_PER_CORE):
                # ---- pass 1: anchor (segment mean) ----
                anchor_ps = anchor_ps_pool.tile([P, D + 1], f32, name=f"anc_ps{w}", tag="anchor_ps")
                for t in range(T_W):
                    g = w * T_W + t
                    oh = oh_pool.tile([P, P], f32, name=f"oh{g}", tag="oh")
                    nc.vector.tensor_scalar(
                        oh[:], iota_sb[:], seg_tiles[g][:], None, Alu.is_equal)
                    nc.tensor.matmul(anchor_ps[:], oh[:], nat_tiles[g][:],
                                     start=(t == 0), stop=(t == T_W - 1))
                cnt = col_pool.tile([P, 1], f32, name=f"cnt{w}", tag="col")
                nc.vector.tensor_scalar(cnt[:], anchor_ps[:, D:D + 1], 1.0, None,
                                        Alu.max)
                rcnt = col_pool.tile([P, 1], f32, name=f"rcnt{w}", tag="col")
                nc.vector.reciprocal(rcnt[:], cnt[:])
                anchor_sb = wnd_pool.tile([P, D], f32, name=f"anch{w}", tag="anchor")
                nc.vector.tensor_scalar(anchor_sb[:], anchor_ps[:, 0:D], rcnt[:],
                                        None, Alu.mult)
                nc.sync.dma_start(out_dram[w, :, D:2 * D], anchor_sb[:])

                # feat_v = anchor @ Wv.T + bv  (via transposed anchor)
                anchT = wnd_pool.tile([P, 2, P], f32, name=f"anchT{w}", tag="anchT")
                for db in range(2):
                    trp = tr_ps_pool.tile([P, P], f32, name=f"atr{w}_{db}",
                                          tag="tr_ps")
                    nc.tensor.transpose(trp[:], anchor_sb[:, db * P:(db + 1) * P],
                                        ident_sb[:])
                    nc.scalar.copy(anchT[:, db, :], trp[:])
                fv_ps = fv_ps_pool.tile([P, D], f32, name=f"fv_ps{w}", tag="fv_ps")
                for db in range(2):
                    nc.tensor.matmul(fv_ps[:], anchT[:, db, :], wvT_sb[:, db, :],
                                     start=(db == 0), stop=(db == 1))
                fv_sb = wnd_pool.tile([P, D], f32, name=f"fv{w}", tag="fv")
                nc.vector.tensor_tensor(fv_sb[:], fv_ps[:], bvb_sb[:], Alu.add)

                # ---- pass 2: logits, segment softmax, weighted sum ----
                wsum_ps = wsum_ps_pool.tile([P, D + 1], f32, name=f"wsum_ps{w}", tag="wsum_ps")
                for t in range(T_W):
                    g = w * T_W + t
                    nat = nat_tiles[g]
                    ifT = ifT_pool.tile([P, 2, P], f32, name=f"ifT{g}", tag="ifT")
                    for db in range(2):
                        trp = tr_ps_pool.tile([P, P], f32, name=f"tr{g}_{db}",
                                              tag="tr_ps")
                        nc.tensor.transpose(trp[:], nat[:, db * P:(db + 1) * P],
                                            ident_sb[:])
                        nc.vector.tensor_copy(ifT[:, db, :], trp[:])
                    oh2 = oh_pool.tile([P, P], f32, name=f"oh2_{g}", tag="oh")
                    nc.vector.tensor_scalar(
                        oh2[:], iota_sb[:], seg_tiles[g][:], None, Alu.is_equal)
                    ohT_ps = tr_ps_pool.tile([P, P], f32, name=f"ohTp{g}",
                                             tag="tr_ps")
                    nc.tensor.transpose(ohT_ps[:], oh2[:], ident_sb[:])
                    ohT = ohT_pool.tile([P, P], f32, name=f"ohT{g}", tag="ohT")
                    nc.scalar.copy(ohT[:], ohT_ps[:])

                    s_ps = s_ps_pool.tile([P, D], f32, name=f"s_ps{g}", tag="s_ps")
                    nc.tensor.matmul(s_ps[:], ifT[:, 0, :], wuT_sb[:, 0, :],
                                     start=True, stop=False)
                    nc.tensor.matmul(s_ps[:], ifT[:, 1, :], wuT_sb[:, 1, :],
                                     start=False, stop=False)
                    nc.tensor.matmul(s_ps[:], ohT[:], fv_sb[:],
                                     start=False, stop=True)
                    s_sb = s_pool.tile([P, D], f32, name=f"s{g}", tag="s")
                    nc.scalar.activation(s_sb[:], s_ps[:], Act.Sigmoid)
                    prod = prod_pool.tile([P, D], f32, name=f"pr{g}", tag="prod")
                    e_col = col_pool.tile([P, 1], f32, name=f"e{g}", tag="col")
                    nc.vector.scalar_tensor_tensor(
                        out=prod[:], in0=s_sb[:], scalar=1.0, in1=web_sb[:],
                        op0=Alu.mult, op1=Alu.mult, accum_out=e_col[:])
                    z_col = col_pool.tile([P, 1], f32, name=f"z{g}", tag="col")
                    nc.scalar.activation(z_col[:], e_col[:], Act.Exp)
                    zx = zx_pool.tile([P, D + 1], f32, name=f"zx{g}", tag="zx")
                    nc.vector.tensor_scalar(zx[:], nat[:], z_col[:], None, Alu.mult)
                    nc.tensor.matmul(wsum_ps[:], oh2[:], zx[:],
                                     start=(t == 0), stop=(t == T_W - 1))
                den = col_pool.tile([P, 1], f32, name=f"den{w}", tag="col")
                nc.vector.tensor_scalar(den[:], wsum_ps[:, D:D + 1], 1e-30, None,
                                        Alu.max)
                rden = col_pool.tile([P, 1], f32, name=f"rden{w}", tag="col")
                nc.vector.reciprocal(rden[:], den[:])
                rst_sb = wnd_pool.tile([P, D], f32, name=f"rst{w}", tag="rst")
                nc.vector.tensor_scalar(rst_sb[:], wsum_ps[:, 0:D], rden[:],
                                        None, Alu.mult)
                nc.sync.dma_start(out_dram[w, :, 0:D], rst_sb[:])

    return nc


def _prepare(ifeat, Wu, Wv, bv, we, seg_ids):
    """Host-side shard + pad. Returns (T_W, in_maps, window node counts)."""
    ifeat = np.asarray(ifeat, dtype=np.float32)
    Wu = np.asarray(Wu, dtype=np.float32)
    Wv = np.asarray(Wv, dtype=np.float32)
    bv = np.asarray(bv, dtype=np.float32)
    we = np.asarray(we, dtype=np.float32)
    seg_ids = np.asarray(seg_ids)

    bounds = np.searchsorted(
        seg_ids, np.arange(0, B + 1, SEGS_PER_WINDOW), side="left")
    n_w = np.diff(bounds)  # nodes per window [16]
    T_W = max(1, int(-(-int(n_w.max()) // P)))

    wuT = np.ascontiguousarray(Wu.T).reshape(2, P, D)
    wvT = np.ascontiguousarray(Wv.T).reshape(2, P, D)
    web = np.tile(we, (P, 1)).astype(np.float32)
    bvb = np.tile(bv, (P, 1)).astype(np.float32)
    ident = np.eye(P, dtype=np.float32)
    iota = np.tile(np.arange(P, dtype=np.float32), (P, 1))

    in_maps = []
    for c in range(N_CORES):
        NT = W_PER_CORE * T_W
        if_pad = np.zeros((NT * P, D + 1), dtype=np.float32)
        if_pad[:, D] = 1.0
        seg_pad = np.full((NT * P,), 500.0, dtype=np.float32)
        for wl in range(W_PER_CORE):
            w = c * W_PER_CORE + wl
            lo, hi = bounds[w], bounds[w + 1]
            base = wl * T_W * P
            if_pad[base:base + (hi - lo), 0:D] = ifeat[lo:hi]
            seg_pad[base:base + (hi - lo)] = (
                seg_ids[lo:hi].astype(np.float32) - w * SEGS_PER_WINDOW)
        in_maps.append({
            "ifeat": if_pad.reshape(NT, P, D + 1),
            "seg": seg_pad.reshape(NT, P, 1),
            "wuT": wuT, "wvT": wvT, "web": web, "bvb": bvb,
            "ident": ident, "iota": iota,
        })
    return T_W, in_maps


_LAST = {}


def _run(ifeat, Wu, Wv, bv, we, seg_ids, trace=False):
    from concourse.bass_utils import run_bass_kernel_spmd

    T_W, in_maps = _prepare(ifeat, Wu, Wv, bv, we, seg_ids)
    nc = _build(T_W)
    _split_sync_waits(nc)
    res = run_bass_kernel_spmd(nc, in_maps, list(range(N_CORES)), trace=trace)
    _LAST["res"] = res
    _LAST["T_W"] = T_W
    _LAST["nc"] = nc
    _LAST["in_maps"] = in_maps

    out = np.empty((B, 2 * D), dtype=np.float32)
    for c in range(N_CORES):
        core_out = res.results[c]["out"]  # [W_PER_CORE, P, 2D]
        for wl in range(W_PER_CORE):
            w = c * W_PER_CORE + wl
            out[w * SEGS_PER_WINDOW:(w + 1) * SEGS_PER_WINDOW, :] = core_out[wl]
    return out


def kernel(ifeat, Wu, Wv, bv, we, seg_ids):
    return _run(ifeat, Wu, Wv, bv, we, seg_ids, trace=False)
